# revision 11
# baseline (speedup 1.0000x reference)
"""Trainium2 Bass kernel for nn_EA_5566277615732 (v3, restructured).

Data-parallel over batch across 8 NeuronCores (32 rows each); parameters
replicated. Host-side prep (pure data reformatting): weights pre-transposed
into lhsT layouts, embedding tables padded with a zero row so pre-masked
token indices (pad -> sentinel row) make the gathers produce already-masked
embeddings, the two 50-dim distance tables merged into one 201x201 product
table (one gather per token instead of two), index tensors pre-transposed.

Device layout: tokens feature-major in four "quarter" tile groups (8 batch
rows each, 130 cols per block with per-block zero borders for the conv).
All heavy matmuls are borderless N=512 (3-D strided rhs views), fp32r.
The attention arg-embedding bias and the 16-feature tail are folded into a
single 48-row matmul per output chunk (rows 0:16 = feature tail, rows
16:48 = per-batch 0/1 selector against CT = argE @ WaArg.T). Work is
pipelined at half-quarter granularity; each quarter's softmax/pooling/dense
tail is traced one half-quarter late so the PE never waits on it.
"""
import numpy as np
from contextlib import ExitStack

import concourse.bass as bass
import concourse.bacc as bacc
import concourse.tile as tile
import concourse.mybir as mybir
from concourse.masks import make_identity

F32 = mybir.dt.float32
F32R = mybir.dt.float32r
I32 = mybir.dt.int32

B, T = 256, 128
NCORES = 8
BC = B // NCORES          # 32 batch rows per core
V, WD, DD, DV = 50000, 300, 50, 200
IN = WD + 2 * DD          # 400
AD = IN + WD              # 700
NF, NCLS = 512, 19
FEAT = NF + 2 * IN        # 1312
DV2 = (DV + 1) * (DV + 1)  # product dist table rows (sentinel = last row)

TS = T + 2                # 130 cols per batch block (with zero borders)
NQ, QB = 4, 8             # 4 quarters x 8 batch rows
W = 1 + QB * TS + 5       # quarter tile width (1046)

OC = [(0, 128), (128, 128), (256, 128), (384, 128), (512, 128), (640, 60)]
WC = [(0, 112), (112, 128), (240, 60)]        # arg-part chunks of WaT rows 400:700
FC = [(0, 128), (128, 128), (256, 128), (384, 128)]
# v-pool feature chunks: (source 0=word/1=dist12, src_offset, size)
VCH = [(0, 0, 128), (0, 128, 128), (0, 256, 44), (1, 0, 50), (1, 50, 50)]

NEG_BIG = 1e30


def _view(ap, h, k):
    """Borderless view: cols (k+1) + 520*h + 130*b + t for b in 0..3, t in 0..127."""
    s = (k + 1) + 520 * h
    return ap[:, s:s + 520].rearrange("p (b t) -> p b t", t=TS)[:, :, 0:T]


def _build_body(nc, tc, ctx, io):
    perm = ctx.enter_context(tc.tile_pool(name="perm", bufs=1))
    gps = ctx.enter_context(tc.tile_pool(name="gps", bufs=2, space="PSUM"))
    cps = ctx.enter_context(tc.tile_pool(name="cps", bufs=2, space="PSUM"))
    aps = ctx.enter_context(tc.tile_pool(name="aps", bufs=2, space="PSUM"))
    sps = ctx.enter_context(tc.tile_pool(name="sps", bufs=2, space="PSUM"))
    gpool = ctx.enter_context(tc.tile_pool(name="gpool", bufs=1))
    tpool = ctx.enter_context(tc.tile_pool(name="tpool", bufs=3))

    ident = perm.tile([128, 128], F32, tag="ident")
    make_identity(nc, ident[:])
    identr = perm.tile([128, 128], F32R, tag="identr")
    nc.vector.tensor_copy(identr[:], ident[:])

    # ---------------- small per-core inputs ----------------
    wsT = perm.tile([128, BC], I32, tag="wsT")
    w12T = perm.tile([128, BC], I32, tag="w12T")
    mask8 = [perm.tile([QB, T], F32, tag=f"mask8_{q}", name=f"mask8_{q}")
             for q in range(NQ)]
    arg1 = perm.tile([BC, 1], I32, tag="arg1")
    arg2 = perm.tile([BC, 1], I32, tag="arg2")
    nc.sync.dma_start(wsT[:], io["wsT"][:])
    nc.sync.dma_start(w12T[:], io["w12T"][:])
    for q in range(NQ):
        nc.sync.dma_start(mask8[q][:], io["wmask"][q * QB:(q + 1) * QB, :])
    nc.sync.dma_start(arg1[:], io["arg1"][:])
    nc.sync.dma_start(arg2[:], io["arg2"][:])

    # gather issue helper: word + fused-dist gathers for one batch row
    def issue_row(q, j):
        b = q * QB + j
        tw = gpool.tile([128, WD], F32R, tag=f"gw{j}", bufs=2, name=f"gw{j}")
        nc.gpsimd.indirect_dma_start(
            out=tw[:], out_offset=None, in_=io["word_emb"][:],
            in_offset=bass.IndirectOffsetOnAxis(ap=wsT[:, b:b + 1], axis=0))
        t12 = gpool.tile([128, 2 * DD], F32R, tag=f"g12_{j}", bufs=2,
                         name=f"g12_{j}")
        nc.gpsimd.indirect_dma_start(
            out=t12[:], out_offset=None, in_=io["dist12"][:],
            in_offset=bass.IndirectOffsetOnAxis(ap=w12T[:, b:b + 1], axis=0))
        return tw, t12

    pending = {}

    def issue_half(q, h):
        if (q, h) not in pending:
            pending[(q, h)] = [issue_row(q, j) for j in range(4 * h, 4 * h + 4)]
        return pending.pop((q, h))

    # prime the pipeline: first half-quarter's gathers go first on the Pool
    # queue so PE work becomes available as early as possible
    pending[(0, 0)] = [issue_row(0, j) for j in range(4)]

    # ---------------- weights (host-packed, plain DMAs) ----------------
    wk = perm.tile([128, 9 * NF], F32R, tag="wk")
    for i in range(9):
        nc.sync.dma_start(wk[:, i * NF:(i + 1) * NF],
                          io["convk"][:, i * NF:(i + 1) * NF])
    wtail = perm.tile([80, NF], F32R, tag="wtail")
    nc.sync.dma_start(wtail[:], io["convt"][:])
    cb = perm.tile([128, 4], F32, tag="cb")
    nc.sync.dma_start(cb[:], io["cb"][:])
    wrT = perm.tile([128, 12], F32R, tag="wrT")
    nc.sync.dma_start(wrT[:], io["wrT"][:])
    db8 = perm.tile([QB, NCLS], F32, tag="db8")
    nc.sync.dma_start(db8[:], io["db"][:].unsqueeze(0).to_broadcast((QB, NCLS)))

    WaT = [[], []]
    argW = [[], []]
    attnT = []
    _argwp_cm = tc.tile_pool(name="argwp", bufs=1)
    argwp = _argwp_cm.__enter__()
    for p in range(2):
        for d in range(3):
            t = perm.tile([128, AD], F32R, tag=f"waT{p}_{d}", name=f"waT{p}_{d}")
            nc.sync.dma_start(t[:], io[f"waT{p + 1}"][128 * d:128 * (d + 1), :])
            WaT[p].append(t)
        at = perm.tile([48, AD], F32R, tag=f"attnT{p}", name=f"attnT{p}")
        nc.sync.dma_start(at[32:48, :], io[f"waT{p + 1}"][384:400, :])
        attnT.append(at)
        for wi, (ws, wz) in enumerate(WC):
            t = argwp.tile([wz, AD], F32R, tag=f"argW{p}_{wi}",
                           name=f"argW{p}_{wi}")
            nc.sync.dma_start(t[:], io[f"waT{p + 1}"][IN + ws:IN + ws + wz, :])
            argW[p].append(t)

    dwT = []
    fchunks = [(fs, fz) for (fs, fz) in FC]
    for p in range(2):
        base = NF + p * IN
        fchunks += [(base + 0, 128), (base + 128, 128), (base + 256, 44),
                    (base + 300, 50), (base + 350, 50)]
    for i, (cs, cz) in enumerate(fchunks):
        t = perm.tile([cz, NCLS], F32, tag=f"dwT{i}", name=f"dwT{i}")
        nc.sync.dma_start(t[:], io["dwT"][cs:cs + cz, :])
        dwT.append(t)

    # softmax NEG addend for pad positions: (mask-1)*BIG
    addend8 = []
    for q in range(NQ):
        t = perm.tile([QB, T], F32, tag=f"addend8_{q}", name=f"addend8_{q}")
        nc.vector.tensor_scalar(out=t[:], in0=mask8[q][:], scalar1=1.0,
                                scalar2=NEG_BIG, op0=mybir.AluOpType.subtract,
                                op1=mybir.AluOpType.mult)
        addend8.append(t)

    # ---------------- big persistent quarter tiles ----------------
    xmB = [[perm.tile([128, W], F32R, tag=f"xmB{q}_{d}", name=f"xmB{q}_{d}")
            for d in range(3)] for q in range(NQ)]
    xq = [perm.tile([80, W], F32R, tag=f"xq{q}", name=f"xq{q}")
          for q in range(NQ)]

    for q in range(NQ):
        for tl in xmB[q]:
            tf = tl[:].bitcast(F32)
            nc.vector.memset(tf[:, 0:1], 0.0)
            nc.vector.memset(tf[:, 1 + QB * TS:W], 0.0)
            v3 = tf[:, 1:1 + QB * TS].rearrange("p (b t) -> p b t", t=TS)
            nc.vector.memset(v3[:, :, 0:1], 0.0)
            nc.vector.memset(v3[:, :, TS - 1:TS], 0.0)
        # tail rows: full memset (borders + gaps); S rows via DMA
        nc.vector.memset(xq[q][32:48, :].bitcast(F32), 0.0)
        nc.sync.dma_start(xq[q][0:32, :], io["sful"][:, q * W:(q + 1) * W])

    featB_cnn = [perm.tile([128, BC], F32, tag=f"fcnn{i}", name=f"fcnn{i}")
                 for i in range(4)]
    featB_v = [[perm.tile([dz, BC], F32, tag=f"fv{p}_{c}", name=f"fv{p}_{c}")
                for c, (src, ds, dz) in enumerate(VCH)] for p in range(2)]

    # ---------------- argE gathers + CT = argE @ WaArg ----------------
    for p, argt in enumerate((arg1, arg2)):
        ea = tpool.tile([BC, WD], F32R, tag=f"argEA{p}", name=f"argEA{p}", bufs=1)
        nc.gpsimd.indirect_dma_start(
            out=ea[:], out_offset=None, in_=io["word_emb"][:],
            in_offset=bass.IndirectOffsetOnAxis(ap=argt[:, 0:1], axis=0))
        argEB = []
        for wi, (ws, wz) in enumerate(WC):
            tp = sps.tile([wz, BC], F32R, space="PSUM", tag="sm", name="argtp")
            nc.tensor.transpose(out=tp[:], in_=ea[:, ws:ws + wz],
                                identity=identr[0:BC, 0:BC])
            t = tpool.tile([wz, BC], F32R, tag=f"argEB{p}_{wi}",
                           name=f"argEB{p}_{wi}", bufs=1)
            nc.vector.tensor_copy(t[:], tp[:])
            argEB.append(t)
        for cs, cz in ((0, 512), (512, AD - 512)):
            cp = sps.tile([BC, 512], F32, space="PSUM", tag="sm", name="ctps")
            for wi in range(3):
                nc.tensor.matmul(cp[:, 0:cz], lhsT=argEB[wi][:],
                                 rhs=argW[p][wi][:, cs:cs + cz],
                                 start=(wi == 0), stop=(wi == 2))
            nc.vector.tensor_copy(attnT[p][0:32, cs:cs + cz], cp[:, 0:cz])
    _argwp_cm.__exit__(None, None, None)

    # quarter tail: softmax + pooling + dense for quarter q (traced late)
    def quarter_tail(q, gw, g12, sc8p):
        b0g = q * QB
        for p in range(2):
            sc8 = sc8p[p]
            s8 = tpool.tile([QB, T], F32, tag="s8")
            nc.vector.tensor_tensor(out=s8[:], in0=sc8[:],
                                    in1=mask8[q][:],
                                    op=mybir.AluOpType.mult)
            nc.vector.tensor_add(s8[:], s8[:], addend8[q][:])
            negmax = tpool.tile([QB, 1], F32, tag="negmax")
            nc.vector.tensor_reduce(out=negmax[:], in_=s8[:],
                                    axis=mybir.AxisListType.X,
                                    op=mybir.AluOpType.max, negate=True)
            e8 = tpool.tile([QB, T], F32, tag="e8")
            esum = tpool.tile([QB, 1], F32, tag="esum")
            nc.scalar.activation(e8[:], s8[:], mybir.ActivationFunctionType.Exp,
                                 bias=negmax[:], accum_out=esum[:])
            rsum = tpool.tile([QB, 1], F32, tag="rsum")
            nc.vector.reciprocal(rsum[:], esum[:])
            anorm = tpool.tile([QB, T], F32, tag="anorm")
            nc.vector.tensor_scalar_mul(anorm[:], e8[:], rsum[:, 0:1])
            atp = sps.tile([128, QB], F32, space="PSUM", tag="sm", name="atp")
            nc.tensor.transpose(out=atp[:], in_=anorm[:],
                                identity=ident[0:QB, 0:QB])
            aT8 = tpool.tile([128, QB], F32, tag="aT8")
            nc.vector.tensor_copy(aT8[:], atp[:])

            srcs = (gw, g12)
            for c, (src, ds, dz) in enumerate(VCH):
                vp = sps.tile([dz, QB], F32, space="PSUM", tag="sm",
                              name=f"vps{p}_{c}")
                for j in range(QB):
                    nc.tensor.matmul(vp[:, j:j + 1],
                                     lhsT=srcs[src][j][:, ds:ds + dz].bitcast(F32),
                                     rhs=aT8[:, j:j + 1], start=True, stop=True)
                nc.vector.tensor_copy(featB_v[p][c][:, b0g:b0g + QB], vp[:])

        for fi in range(4):
            nc.scalar.activation(featB_cnn[fi][:, b0g:b0g + QB],
                                 featB_cnn[fi][:, b0g:b0g + QB],
                                 mybir.ActivationFunctionType.Tanh,
                                 bias=cb[:, fi:fi + 1])

        lg = sps.tile([QB, NCLS], F32, space="PSUM", tag="sm", name="lg")
        featB = featB_cnn + featB_v[0] + featB_v[1]
        for i, ft in enumerate(featB):
            nc.tensor.matmul(lg[:], lhsT=ft[:, b0g:b0g + QB], rhs=dwT[i][:],
                             start=(i == 0), stop=(i == len(featB) - 1))
        lgs = tpool.tile([QB, NCLS], F32, tag="lgs")
        nc.vector.tensor_add(lgs[:], lg[:], db8[:])
        lmax = tpool.tile([QB, 1], F32, tag="lmax")
        nc.vector.tensor_reduce(out=lmax[:], in_=lgs[:],
                                axis=mybir.AxisListType.X,
                                op=mybir.AluOpType.max, negate=True)
        le = tpool.tile([QB, NCLS], F32, tag="le")
        lsum = tpool.tile([QB, 1], F32, tag="lsum")
        nc.scalar.activation(le[:], lgs[:], mybir.ActivationFunctionType.Exp,
                             bias=lmax[:], accum_out=lsum[:])
        lrs = tpool.tile([QB, 1], F32, tag="lrs")
        nc.vector.reciprocal(lrs[:], lsum[:])
        osb = tpool.tile([QB, NCLS], F32, tag="osb")
        nc.vector.tensor_scalar_mul(osb[:], le[:], lrs[:, 0:1])
        nc.sync.dma_start(io["out"][b0g:b0g + QB, :], osb[:])

    # ---------------- main pipeline, half-quarter granularity ----------------
    deferred = None
    attn_def = None
    for q in range(NQ):
        b0g = q * QB
        gw, g12 = [None] * QB, [None] * QB
        sc8p = [tpool.tile([QB, T], F32, tag=f"sc8_{p}", name=f"sc8_{p}")
                for p in range(2)]

        for h in range(2):
            for j, (tw, t12) in zip(range(4 * h, 4 * h + 4), issue_half(q, h)):
                gw[j], g12[j] = tw, t12
            # issue the next half-quarter's gathers (one half ahead)
            nxt = (q, 1) if h == 0 else (q + 1, 0)
            if nxt[0] < NQ:
                pending[nxt] = [issue_row(nxt[0], j)
                                for j in range(4 * nxt[1], 4 * nxt[1] + 4)]

            # ---- transpose to feature-major (4 rows per PSUM tile) ----
            def tp4(srcs, ds, dz):
                tp = gps.tile([128, 512], F32R, space="PSUM", tag="g", name="gtp")
                for j in range(4):
                    nc.tensor.transpose(out=tp[0:dz, 128 * j:128 * (j + 1)],
                                        in_=srcs[4 * h + j][:, ds:ds + dz],
                                        identity=identr[:])
                return tp

            def dview(dst, r0, rz):
                return dst[r0:r0 + rz, 2 + 520 * h:2 + 520 * h + 520] \
                    .rearrange("p (b t) -> p b t", t=TS)[:, :, 0:T]

            tp = tp4(gw, 0, 128)
            nc.vector.tensor_copy(dview(xmB[q][0], 0, 128),
                                  tp[:].rearrange("p (b t) -> p b t", t=T))
            tp = tp4(gw, 128, 128)
            nc.vector.tensor_copy(dview(xmB[q][1], 0, 128),
                                  tp[:].rearrange("p (b t) -> p b t", t=T))
            tp = tp4(gw, 256, 44)
            nc.vector.tensor_copy(dview(xmB[q][2], 0, 44),
                                  tp[0:44, :].rearrange("p (b t) -> p b t", t=T))
            tp = tp4(g12, 0, 50)
            stg1 = tpool.tile([50, 512], F32R, tag="stgd1", bufs=2, name="stgd1")
            nc.vector.tensor_copy(stg1[:], tp[0:50, :])
            nc.sync.dma_start(dview(xmB[q][2], 44, 50),
                              stg1[:].rearrange("p (b t) -> p b t", t=T))
            tp = tp4(g12, 50, 50)
            stg2 = tpool.tile([50, 512], F32R, tag="stgd2", bufs=2, name="stgd2")
            nc.vector.tensor_copy(stg2[:], tp[0:50, :])
            nc.sync.dma_start(dview(xmB[q][2], 94, 34),
                              stg2[0:34, :].rearrange("p (b t) -> p b t", t=T))
            nc.sync.dma_start(dview(xq[q], 32, 16),
                              stg2[34:50, :].rearrange("p (b t) -> p b t", t=T))

            # shifted tail copies for conv taps k=0 / k=2 (this half's range)
            nc.sync.dma_start(xq[q][48:64, 1 + 520 * h:521 + 520 * h],
                              xq[q][32:48, 520 * h:520 + 520 * h])
            nc.sync.dma_start(xq[q][64:80, 1 + 520 * h:521 + 520 * h],
                              xq[q][32:48, 2 + 520 * h:522 + 520 * h])

            # ---- conv, this half ----
            for fi, (fs, fz) in enumerate(FC):
                pv = cps.tile([128, 512], F32, space="PSUM", tag="cv",
                              name="convps")
                mms = [(wk[:, (3 * k + cc) * NF + fs:(3 * k + cc) * NF + fs + fz],
                        _view(xmB[q][cc][:], h, k))
                       for k in range(3) for cc in range(3)]
                mms.append((wtail[:, fs:fs + fz], _view(xq[q][0:80], h, 1)))
                for i, (lhsT, rhs) in enumerate(mms):
                    nc.tensor.matmul(pv[:, 0:512], lhsT=lhsT, rhs=rhs,
                                     start=(i == 0), stop=(i == len(mms) - 1))
                nc.vector.tensor_reduce(
                    out=featB_cnn[fi][:, b0g + 4 * h:b0g + 4 * h + 4],
                    in_=pv[:].rearrange("p (b t) -> p b t", t=T),
                    axis=mybir.AxisListType.X, op=mybir.AluOpType.max)

            # ---- attention pre + scores (deferred one half) ----
            def attn_half(q=q, h=h, sc8p=sc8p):
                for p in range(2):
                    tts = []
                    for oc, (os_, oz) in enumerate(OC):
                        pre = aps.tile([128, 512], F32, space="PSUM", tag="pre",
                                       name="prepsum")
                        mms = [(WaT[p][d][:, os_:os_ + oz],
                                _view(xmB[q][d][:], h, 1)) for d in range(3)]
                        mms.append((attnT[p][:, os_:os_ + oz],
                                    _view(xq[q][0:48], h, 1)))
                        for i, (lhsT, rhs) in enumerate(mms):
                            nc.tensor.matmul(pre[0:oz, 0:512], lhsT=lhsT,
                                             rhs=rhs, start=(i == 0),
                                             stop=(i == len(mms) - 1))
                        tt = tpool.tile([128, 512], F32R, tag="ttile", bufs=7)
                        nc.scalar.activation(tt[0:oz, :], pre[0:oz, 0:512],
                                             mybir.ActivationFunctionType.Tanh)
                        tts.append(tt)
                    spsum = sps.tile([1, 512], F32, space="PSUM", tag="sm",
                                     name="spsum")
                    for oc, (os_, oz) in enumerate(OC):
                        nc.tensor.matmul(
                            spsum[:, 0:512],
                            lhsT=wrT[0:oz, 6 * p + oc:6 * p + oc + 1],
                            rhs=tts[oc][0:oz, :],
                            start=(oc == 0), stop=(oc == 5))
                    srow = tpool.tile([1, 512], F32, tag="srow", bufs=2)
                    nc.vector.tensor_copy(srow[:], spsum[:, 0:512])
                    nc.sync.dma_start(sc8p[p][4 * h:4 * h + 4, :], srow[:])

            if attn_def is not None:
                attn_def()
            if h == 0 and deferred is not None:
                deferred()
                deferred = None
            attn_def = attn_half

        def mk_tail(q=q, gw=gw, g12=g12, sc8p=sc8p):
            return lambda: quarter_tail(q, gw, g12, sc8p)
        deferred = mk_tail()

    attn_def()
    deferred()


_CACHED = None


def _build():
    global _CACHED
    if _CACHED is not None:
        return _CACHED
    nc = bacc.Bacc("TRN2", target_bir_lowering=False, debug=False,
                   num_devices=NCORES)
    io = {}

    def din(name, shape, dt):
        io[name] = nc.dram_tensor(name, shape, dt, kind="ExternalInput").ap()

    din("wsT", [128, BC], I32)
    din("w12T", [128, BC], I32)
    din("wmask", [BC, T], F32)
    din("arg1", [BC, 1], I32)
    din("arg2", [BC, 1], I32)
    din("word_emb", [V + 1, WD], F32R)
    din("dist12", [DV2, 2 * DD], F32R)
    din("waT1", [AD, AD], F32R)
    din("waT2", [AD, AD], F32R)
    din("wrT", [128, 12], F32R)
    din("convk", [128, 9 * NF], F32R)
    din("convt", [80, NF], F32R)
    din("cb", [128, 4], F32)
    din("dwT", [FEAT, NCLS], F32)
    din("db", [NCLS], F32)
    din("sful", [BC, NQ * W], F32R)
    io["out"] = nc.dram_tensor("out", [BC, NCLS], F32, kind="ExternalOutput").ap()

    with tile.TileContext(nc) as tc:
        with ExitStack() as ctx:
            _build_body(nc, tc, ctx, io)
    nc.compile()
    _CACHED = nc
    return nc


def _pack_shared(inputs):

    def f32(x):
        return np.ascontiguousarray(np.asarray(x), dtype=np.float32)

    we = f32(inputs["word_emb"])
    d1 = f32(inputs["dist1_emb"])
    d2 = f32(inputs["dist2_emb"])
    d1P = np.concatenate([d1, np.zeros((1, DD), np.float32)], 0)
    d2P = np.concatenate([d2, np.zeros((1, DD), np.float32)], 0)
    d12 = np.empty((DV + 1, DV + 1, 2 * DD), np.float32)
    d12[:, :, 0:DD] = d1P[:, None, :]
    d12[:, :, DD:2 * DD] = d2P[None, :, :]
    rep = {
        "word_emb": np.concatenate([we, np.zeros((1, WD), np.float32)], 0),
        "dist12": d12.reshape(DV2, 2 * DD),
        "waT1": np.ascontiguousarray(f32(inputs["Wa1"]).T),
        "waT2": np.ascontiguousarray(f32(inputs["Wa2"]).T),
        "db": f32(inputs["dense_b"]),
        "dwT": np.ascontiguousarray(f32(inputs["dense_w"]).T),
        "cb": np.ascontiguousarray(f32(inputs["conv_b"]).reshape(4, 128).T),
    }
    wrT = np.zeros((128, 12), np.float32)
    for p, wr in enumerate((inputs["wr1"], inputs["wr2"])):
        wr = f32(wr)
        for oc, (os_, oz) in enumerate(OC):
            wrT[0:oz, 6 * p + oc] = wr[os_:os_ + oz]
    rep["wrT"] = wrT
    cw = f32(inputs["conv_w"])                      # [NF, IN, 3]
    convk = np.zeros((128, 9 * NF), np.float32)
    for k in range(3):
        for cc in range(3):
            convk[:, (3 * k + cc) * NF:(3 * k + cc + 1) * NF] = \
                cw[:, cc * 128:cc * 128 + 128, k].T
    rep["convk"] = convk
    convt = np.zeros((80, NF), np.float32)
    convt[32:48] = cw[:, 384:400, 1].T
    convt[48:64] = cw[:, 384:400, 0].T
    convt[64:80] = cw[:, 384:400, 2].T
    rep["convt"] = convt
    sful = np.zeros((BC, NQ, W), np.float32)
    for q in range(NQ):
        for lb in range(QB):
            sful[q * QB + lb, q, 2 + TS * lb:2 + TS * lb + T] = 1.0
    rep["sful"] = sful.reshape(BC, NQ * W)
    return rep


def kernel(trace=False, **inputs):
    nc = _build()
    from concourse.bass_utils import run_bass_kernel_spmd

    def i32(x):
        return np.ascontiguousarray(np.asarray(x), dtype=np.int32)

    def f32(x):
        return np.ascontiguousarray(np.asarray(x), dtype=np.float32)

    rep = _pack_shared(inputs)
    wm = f32(inputs["words_mask"])
    keep = wm > 0
    wsM = np.where(keep, i32(inputs["words_seq"]), V).astype(np.int32)
    w1M = i32(inputs["words_arg1_dist_seq"])
    w2M = i32(inputs["words_arg2_dist_seq"])
    w12M = np.where(keep, w1M * (DV + 1) + w2M, DV2 - 1).astype(np.int32)
    a1 = i32(inputs["arg1"]).reshape(B, 1)
    a2 = i32(inputs["arg2"]).reshape(B, 1)

    in_maps = []
    for c in range(NCORES):
        sl = slice(c * BC, (c + 1) * BC)
        m = dict(rep)
        m.update(wsT=np.ascontiguousarray(wsM[sl].T),
                 w12T=np.ascontiguousarray(w12M[sl].T),
                 wmask=wm[sl], arg1=a1[sl], arg2=a2[sl])
        in_maps.append(m)

    res = run_bass_kernel_spmd(nc, in_maps, core_ids=list(range(NCORES)),
                               trace=trace)
    out = np.concatenate([res.results[c]["out"] for c in range(NCORES)], axis=0)
    if trace:
        return out.astype(np.float32), res
    return out.astype(np.float32)
